# revision 13
# baseline (speedup 1.0000x reference)
"""Trainium2 Bass kernel for nn_Loss_50233937494630 (chamfer+arap+aux loss).

Sharding (8 cores, data-parallel per the hint): core c handles batch b = c//2,
row-half h = c%2 (rows [2048h, 2048h+2048) of N=4096).

Phase A (one SPMD Bass program on all 8 cores):
  - All pairwise-distance maps are computed on the PE via an augmented K=5
    matmul in float32r producing the NEGATED squared-distance map directly:
      lhsT = [p0, p1, p2, |p|^2, 1], rhs = [2q0, 2q1, 2q2, -1, -|q|^2]
      => (lhsT.T @ rhs)[m, n] = -(|p_m - q_n|^2)
  - Chamfer per t: orientation A (rows = dp half, cols = target) gives d1 via a
    fused tensor_tensor_reduce over slab halves (ACT pre-copies one half of
    each PSUM slab to SBUF so the DVE ingests two fresh elements per cycle);
    orientation B (rows = target, cols = dp half) gives d2 partials.
  - 8-NN of source points: source map with per-core ROTATED columns (so the
    self-diagonal sits at static column blocks on every core), diagonal masked
    with -BIG, then DVE max (top-8) + max_index.  sd = sqrt(-v + 1e-5).
  - pd / sp / tran partials with tiny reductions.

Host: combines partial sums/mins across cores and gathers dp[idx] (pure data
movement).  Phase B (tiny kernel): arap loss from gathered neighbour coords.
"""

import sys
from contextlib import ExitStack

import numpy as np

sys.path.insert(0, "/opt/trn_rl_repo")

import concourse.bass as bass  # noqa: E402
import concourse.mybir as mybir  # noqa: E402
import concourse.tile as tile  # noqa: E402
from concourse import bacc  # noqa: E402
from concourse.bass_utils import run_bass_kernel_spmd  # noqa: E402

T, B, N = 3, 4, 4096
HALF = N // 2
KNN = 8
NT_H = HALF // 128  # 16 row-tiles per core
NT_F = N // 128  # 32 row-tiles over a full point set
NCHUNK = 512
BIG = 1e30
F32 = mybir.dt.float32
MAPDT = mybir.dt.float32r  # PE map matmul dtype (1 cyc/row at N=512)
AluOp = mybir.AluOpType
Act = mybir.ActivationFunctionType
AxX = mybir.AxisListType.X

_CACHE = {}
_LAST_INMAPS_A = None
_LAST_INMAPS_B = None


def _r(ap):
    if MAPDT == F32:
        return ap
    return ap.bitcast(MAPDT)


def _build_planes(nc, pool, raw_ap, npts, name):
    """raw_ap: DRAM [npts, 3] f32 -> DRAM plane [10, npts]:
    rows 0-2 coords, 3 |p|^2, 4 ones  (lhsT form = rows 0:5)
    rows 5-7 2*coords, 8 -1, 9 -|p|^2 (rhs form = rows 5:10)"""
    npart = npts // 32
    pp = pool.tile([npart, 96], F32, tag="ppload", name=f"pp_{name}")
    nc.sync.dma_start(pp[:], raw_ap.rearrange("(p j) c -> p (j c)", j=32))
    f = pool.tile([npart, 320], F32, tag="fbuild", name=f"f_{name}")
    fv = f[:].rearrange("p (f j) -> p f j", j=32)
    ppv = pp[:].rearrange("p (j c) -> p j c", c=3)
    for c in range(3):
        nc.vector.tensor_scalar(fv[:, c, :], ppv[:, :, c], 1.0, None, op0=AluOp.mult)
        nc.vector.tensor_scalar(fv[:, 5 + c, :], ppv[:, :, c], 2.0, None,
                                op0=AluOp.mult)
    sq = pool.tile([npart, 96], F32, tag="sqbuild", name=f"sq_{name}")
    nc.vector.tensor_tensor(sq[:], pp[:], pp[:], op=AluOp.mult)
    nc.vector.tensor_reduce(fv[:, 3, :], sq[:].rearrange("p (j c) -> p j c", c=3),
                            axis=AxX, op=AluOp.add)
    nc.vector.memset(fv[:, 4, :], 1.0)
    nc.vector.memset(fv[:, 8, :], -1.0)
    nc.vector.tensor_scalar(fv[:, 9, :], fv[:, 3, :], -1.0, None, op0=AluOp.mult)
    pl_dram = nc.dram_tensor(f"pldram_{name}", [10, npts], F32,
                             kind="Internal").ap()
    nc.sync.dma_start(pl_dram.rearrange("f (p j) -> p f j", j=32), fv)
    return pl_dram


def _lhs_tile(nc, pool, pl_dram, npts, tag):
    t = pool.tile([5, npts], F32, tag=tag, name=tag)
    nc.sync.dma_start(t[:], pl_dram[0:5, :])
    return t


def _rhs_tile(nc, pool, pl_dram, npts, tag):
    t = pool.tile([5, npts], F32, tag=tag, name=tag)
    nc.sync.dma_start(t[:], pl_dram[5:10, :])
    return t


def _map_tile(nc, psum, lhsT_slice, rhs_tile, col0, ncols, exact=False):
    """matmul a [128, ncols] negated-distance slab into a fresh PSUM tile"""
    cast = (lambda ap: ap) if exact else _r
    slab = psum.tile([128, ncols], F32, tag="slab", name="slab")
    for j in range(ncols // NCHUNK):
        nc.tensor.matmul(
            slab[:, j * NCHUNK:(j + 1) * NCHUNK],
            cast(lhsT_slice),
            cast(rhs_tile[:, col0 + j * NCHUNK:col0 + (j + 1) * NCHUNK]),
            start=True, stop=True,
        )
    return slab


def _build_kernel_a():
    nc = bacc.Bacc("TRN2", target_bir_lowering=False, debug=False)
    src_rot = nc.dram_tensor("src_rot", [N, 3], F32, kind="ExternalInput").ap()
    src_half = nc.dram_tensor("src_half", [HALF, 3], F32, kind="ExternalInput").ap()
    tgt_full = nc.dram_tensor("tgt_full", [N, 3], F32, kind="ExternalInput").ap()
    dp_half = nc.dram_tensor("dp_half", [T, HALF, 3], F32, kind="ExternalInput").ap()
    drp_half = nc.dram_tensor("drp_half", [T, HALF, 3], F32,
                              kind="ExternalInput").ap()
    pw_half = nc.dram_tensor("pw_half", [T, HALF], F32, kind="ExternalInput").ap()
    rigid3 = nc.dram_tensor("rigid3", [1, T * 3], F32, kind="ExternalInput").ap()

    o_d1 = nc.dram_tensor("d1sums", [128, T], F32, kind="ExternalOutput").ap()
    o_d2 = nc.dram_tensor("d2part", [128, T * NT_F], F32, kind="ExternalOutput").ap()
    o_sd = nc.dram_tensor("sd", [128, NT_H * KNN], F32, kind="ExternalOutput").ap()
    o_idx = nc.dram_tensor("idx", [128, NT_H * KNN], mybir.dt.uint32,
                           kind="ExternalOutput").ap()
    o_pd = nc.dram_tensor("pdpart", [128, T], F32, kind="ExternalOutput").ap()
    o_sp = nc.dram_tensor("sppart", [128, T], F32, kind="ExternalOutput").ap()
    o_tr = nc.dram_tensor("tranpart", [1, T], F32, kind="ExternalOutput").ap()

    with tile.TileContext(nc) as tc, ExitStack() as ctx:
        aug = ctx.enter_context(tc.tile_pool(name="aug", bufs=1))
        psum = ctx.enter_context(tc.tile_pool(name="psum", bufs=2, space="PSUM"))

        # ---------- stage 0: augmented planes & operand tiles ----------
        with tc.tile_pool(name="planes", bufs=1) as plp:
            pl_srcF = _build_planes(nc, plp, src_rot, N, "srcF")
            pl_srcH = _build_planes(nc, plp, src_half, HALF, "srcH")
            pl_tgt = _build_planes(nc, plp, tgt_full, N, "tgt")
            pl_dp = [_build_planes(nc, plp, dp_half[t], HALF, f"dp{t}")
                     for t in range(T)]

            lhs_src = _lhs_tile(nc, aug, pl_srcH, HALF, "lhs_src")
            rhs_src = _rhs_tile(nc, aug, pl_srcF, N, "rhs_src")
            lhs_tgt = _lhs_tile(nc, aug, pl_tgt, N, "lhs_tgt")
            rhs_tgt = _rhs_tile(nc, aug, pl_tgt, N, "rhs_tgt")
            lhs_dp = [_lhs_tile(nc, aug, pl_dp[t], HALF, f"lhs_dp{t}")
                      for t in range(T)]
            rhs_dp = [_rhs_tile(nc, aug, pl_dp[t], HALF, f"rhs_dp{t}")
                      for t in range(T)]

        work = ctx.enter_context(tc.tile_pool(name="work", bufs=2))
        accp = ctx.enter_context(tc.tile_pool(name="accs", bufs=1))
        consts = ctx.enter_context(tc.tile_pool(name="consts", bufs=1))
        negbig_full = consts.tile([128, 128], F32)
        nc.vector.memset(negbig_full[:], -BIG)
        diag = consts.tile([128, 128], F32)
        nc.gpsimd.affine_select(
            diag[:], negbig_full[:], pattern=[[-1, 128]],
            compare_op=AluOp.is_equal, fill=0.0, base=0, channel_multiplier=1,
        )

        d1buf = accp.tile([128, T * NT_H], F32)
        d2buf = accp.tile([128, T * NT_F], F32)
        v8buf = accp.tile([128, NT_H * KNN], F32)
        idxbuf = accp.tile([128, NT_H * KNN], mybir.dt.uint32)

        # ---------- stage 1: spsp map + top-8 ----------
        for g in range(NT_H):
            lw = lhs_src[:, g * 128:(g + 1) * 128]
            ssb = work.tile([128, N], F32, tag="ssb", name="ssb")
            for half in range(2):
                slab = _map_tile(nc, psum, lw, rhs_src, half * HALF, HALF,
                                 exact=True)
                nc.scalar.copy(ssb[:, half * HALF:(half + 1) * HALF], slab[:])
            # self-exclusion: with rotated cols, row p of tile g selfs at col 128g+p
            nc.vector.tensor_tensor(
                ssb[:, g * 128:(g + 1) * 128],
                ssb[:, g * 128:(g + 1) * 128], diag[:], op=AluOp.add,
            )
            nc.vector.max(v8buf[:, g * KNN:(g + 1) * KNN], ssb[:])
            nc.vector.max_index(
                idxbuf[:, g * KNN:(g + 1) * KNN],
                v8buf[:, g * KNN:(g + 1) * KNN], ssb[:],
            )
        sdbuf = accp.tile([128, NT_H * KNN], F32)
        epst = consts.tile([128, 1], F32)
        nc.vector.memset(epst[:], 1e-5)
        vcl = accp.tile([128, NT_H * KNN], F32)
        nc.vector.tensor_scalar(vcl[:], v8buf[:], 0.0, None, op0=AluOp.min)
        nc.scalar.activation(sdbuf[:], vcl[:], Act.Sqrt, bias=epst[:], scale=-1.0)
        nc.sync.dma_start(o_sd, sdbuf[:])
        nc.sync.dma_start(o_idx, idxbuf[:])

        # ---------- stage 2: chamfer (maps hold -d; min d == -max(-d)) ----------
        for t in range(T):
            # orientation A: rows = dp half -> d1 row-maxes
            for g in range(NT_H):
                lw = lhs_dp[t][:, g * 128:(g + 1) * 128]
                acc2 = work.tile([128, 2], F32, tag="acc2", name="acc2")
                for half in range(2):
                    slab = _map_tile(nc, psum, lw, rhs_tgt, half * HALF, HALF)
                    nc.vector.tensor_reduce(acc2[:, half:half + 1], slab[:],
                                            axis=AxX, op=AluOp.max)
                nc.vector.tensor_tensor(
                    d1buf[:, t * NT_H + g:t * NT_H + g + 1],
                    acc2[:, 0:1], acc2[:, 1:2], op=AluOp.max,
                )
            # orientation B: rows = target -> d2 partials (min over our dp half)
            for g in range(NT_F):
                lw = lhs_tgt[:, g * 128:(g + 1) * 128]
                slab = _map_tile(nc, psum, lw, rhs_dp[t], 0, HALF)
                nc.vector.tensor_reduce(d2buf[:, t * NT_F + g:t * NT_F + g + 1],
                                        slab[:], axis=AxX, op=AluOp.max)

        nd1 = accp.tile([128, T * NT_H], F32)
        nc.vector.tensor_scalar(nd1[:], d1buf[:], -1.0, None, op0=AluOp.mult)
        d1s = accp.tile([128, T], F32)
        nc.vector.tensor_reduce(d1s[:], nd1[:].rearrange("p (t g) -> p t g", g=NT_H),
                                axis=AxX, op=AluOp.add)
        nc.sync.dma_start(o_d1, d1s[:])
        nd2 = accp.tile([128, T * NT_F], F32)
        nc.vector.tensor_scalar(nd2[:], d2buf[:], -1.0, None, op0=AluOp.mult)
        nc.sync.dma_start(o_d2, nd2[:])

        # ---------- stage 3: small terms ----------
        small = ctx.enter_context(tc.tile_pool(name="small", bufs=1))
        pdbuf = small.tile([128, T], F32)
        spbuf = small.tile([128, T], F32)
        for t in range(T):
            a = small.tile([128, 48], F32, tag="pd_a", name="pd_a")
            bb = small.tile([128, 48], F32, tag="pd_b", name="pd_b")
            nc.sync.dma_start(a[:], drp_half[t].rearrange("(p j) c -> p (j c)", p=128))
            nc.sync.dma_start(bb[:], dp_half[t].rearrange("(p j) c -> p (j c)", p=128))
            dterm = small.tile([128, 48], F32, tag="pd_d", name="pd_d")
            nc.vector.tensor_tensor(dterm[:], a[:], bb[:], op=AluOp.subtract)
            dsq = small.tile([128, 48], F32, tag="pd_j", name="pd_j")
            nc.vector.tensor_tensor(dsq[:], dterm[:], dterm[:], op=AluOp.mult)
            nc.vector.tensor_reduce(pdbuf[:, t:t + 1], dsq[:], axis=AxX,
                                    op=AluOp.add)
            pwt = small.tile([128, 16], F32, tag="pw", name="pw")
            nc.sync.dma_start(pwt[:], pw_half[t].rearrange("(p j) -> p j", p=128))
            nc.vector.tensor_reduce(spbuf[:, t:t + 1], pwt[:], axis=AxX,
                                    op=AluOp.add, apply_absolute_value=True)
        nc.sync.dma_start(o_pd, pdbuf[:])
        nc.sync.dma_start(o_sp, spbuf[:])
        rg = small.tile([1, T * 3], F32)
        nc.sync.dma_start(rg[:], rigid3)
        rsq = small.tile([1, T * 3], F32)
        nc.vector.tensor_tensor(rsq[:], rg[:], rg[:], op=AluOp.mult)
        trb = small.tile([1, T], F32)
        nc.vector.tensor_reduce(trb[:], rsq[:].rearrange("p (t c) -> p t c", c=3),
                                axis=AxX, op=AluOp.add)
        nc.sync.dma_start(o_tr, trb[:])

    nc.compile()
    return nc


def _build_kernel_b():
    nc = bacc.Bacc("TRN2", target_bir_lowering=False, debug=False)
    dpn = nc.dram_tensor("dpn", [T, HALF * KNN * 3], F32, kind="ExternalInput").ap()
    dpm = nc.dram_tensor("dpm", [T, HALF * KNN * 3], F32, kind="ExternalInput").ap()
    spn = nc.dram_tensor("spn", [HALF * KNN * 3], F32, kind="ExternalInput").ap()
    spm = nc.dram_tensor("spm", [HALF * KNN * 3], F32, kind="ExternalInput").ap()
    o_ar = nc.dram_tensor("arpart", [128, T], F32, kind="ExternalOutput").ap()
    FW = NT_H * KNN * 3  # 384 per partition per t
    with tile.TileContext(nc) as tc, ExitStack() as ctx:
        pool = ctx.enter_context(tc.tile_pool(name="p", bufs=1))
        arbuf = pool.tile([128, T], F32)
        epst = pool.tile([128, 1], F32)
        nc.vector.memset(epst[:], 1e-5)
        # sd from gathered source coords, matching the reference computation
        snt = pool.tile([128, FW], F32)
        nc.sync.dma_start(snt[:], spn.rearrange("(p w) -> p w", p=128))
        smt = pool.tile([128, FW], F32)
        nc.sync.dma_start(smt[:], spm.rearrange("(p w) -> p w", p=128))
        sn = pool.tile([128, FW], F32)
        nc.vector.tensor_tensor(sn[:], snt[:], smt[:], op=AluOp.subtract)
        sn2 = pool.tile([128, FW], F32)
        nc.vector.tensor_tensor(sn2[:], sn[:], sn[:], op=AluOp.mult)
        ssum = pool.tile([128, NT_H * KNN], F32)
        nc.vector.tensor_reduce(ssum[:], sn2[:].rearrange("p (w c) -> p w c", c=3),
                                axis=AxX, op=AluOp.add)
        sdt = pool.tile([128, NT_H * KNN], F32)
        nc.scalar.activation(sdt[:], ssum[:], Act.Sqrt, bias=epst[:], scale=1.0)
        for t in range(T):
            nt = pool.tile([128, FW], F32, tag="nt", name="nt")
            nc.sync.dma_start(nt[:], dpn[t].rearrange("(p w) -> p w", p=128))
            mt = pool.tile([128, FW], F32, tag="mt", name="mt")
            nc.sync.dma_start(mt[:], dpm[t].rearrange("(p w) -> p w", p=128))
            dn = pool.tile([128, FW], F32, tag="dn", name="dn")
            nc.vector.tensor_tensor(dn[:], nt[:], mt[:], op=AluOp.subtract)
            dn2 = pool.tile([128, FW], F32, tag="dn2", name="dn2")
            nc.vector.tensor_tensor(dn2[:], dn[:], dn[:], op=AluOp.mult)
            ss = pool.tile([128, NT_H * KNN], F32, tag="ss", name="ss")
            nc.vector.tensor_reduce(ss[:], dn2[:].rearrange("p (w c) -> p w c", c=3),
                                    axis=AxX, op=AluOp.add)
            dd = pool.tile([128, NT_H * KNN], F32, tag="dd", name="dd")
            nc.scalar.activation(dd[:], ss[:], Act.Sqrt, bias=epst[:], scale=1.0)
            df = pool.tile([128, NT_H * KNN], F32, tag="df", name="df")
            nc.vector.tensor_tensor(df[:], dd[:], sdt[:], op=AluOp.subtract)
            df2 = pool.tile([128, NT_H * KNN], F32, tag="dj", name="dj")
            nc.vector.tensor_tensor(df2[:], df[:], df[:], op=AluOp.mult)
            nc.vector.tensor_reduce(arbuf[:, t:t + 1], df2[:], axis=AxX,
                                    op=AluOp.add)
        nc.sync.dma_start(o_ar, arbuf[:])
    nc.compile()
    return nc


def kernel(phi, point_weight, deform_rigid_points, deformation_points,
           rigid_matrix, source_points, target_points):
    pw = np.asarray(point_weight, dtype=np.float32).reshape(T, B, N)
    drp = np.asarray(deform_rigid_points, dtype=np.float32)
    dp = np.asarray(deformation_points, dtype=np.float32)
    rmx = np.asarray(rigid_matrix, dtype=np.float32)
    src = np.asarray(source_points, dtype=np.float32)
    tgt = np.asarray(target_points, dtype=np.float32)

    if "A" not in _CACHE:
        _CACHE["A"] = _build_kernel_a()
    ncA = _CACHE["A"]

    in_maps = []
    for c in range(8):
        b, h = c // 2, c % 2
        r0 = HALF * h
        in_maps.append({
            "src_rot": np.ascontiguousarray(np.roll(src[b], -r0, axis=0)),
            "src_half": np.ascontiguousarray(src[b, r0:r0 + HALF]),
            "tgt_full": np.ascontiguousarray(tgt[b]),
            "dp_half": np.ascontiguousarray(dp[:, b, r0:r0 + HALF]),
            "drp_half": np.ascontiguousarray(drp[:, b, r0:r0 + HALF]),
            "pw_half": np.ascontiguousarray(pw[:, b, r0:r0 + HALF]),
            "rigid3": np.ascontiguousarray(rmx[:, b, 0:3, 3].reshape(1, T * 3)),
        })
    global _LAST_INMAPS_A
    _LAST_INMAPS_A = in_maps
    resA = run_bass_kernel_spmd(ncA, in_maps, core_ids=list(range(8)))
    outsA = resA.results

    d1 = np.zeros(T, dtype=np.float64)
    d2 = np.zeros(T, dtype=np.float64)
    pd = 0.0
    sp = np.zeros(T, dtype=np.float64)
    tran = 0.0
    for b in range(B):
        c0, c1 = outsA[2 * b], outsA[2 * b + 1]
        d1 += c0["d1sums"].sum(axis=0) + c1["d1sums"].sum(axis=0)
        dm = np.minimum(c0["d2part"], c1["d2part"])  # [128, T*32]
        d2 += dm.reshape(128, T, NT_F).sum(axis=(0, 2))
        pd += c0["pdpart"].sum() + c1["pdpart"].sum()
        sp += c0["sppart"].sum(axis=0) + c1["sppart"].sum(axis=0)
        tran += c0["tranpart"].sum()  # h=0 core only
    total = ((d1 + d2) * 0.5 / B).sum() + pd / B + (sp / (B * N)).sum() + tran / B

    # ---- phase B: arap (host gather = data movement only) ----
    if "B" not in _CACHE:
        _CACHE["B"] = _build_kernel_b()
    ncB = _CACHE["B"]
    in_maps_b = []
    for c in range(8):
        b, h = c // 2, c % 2
        r0 = HALF * h
        idx = outsA[c]["idx"].reshape(128, NT_H, KNN).transpose(1, 0, 2) \
            .reshape(HALF, KNN).astype(np.int64)
        idx = (idx + r0) % N  # un-rotate column space
        sd = outsA[c]["sd"].reshape(128, NT_H, KNN).transpose(1, 0, 2) \
            .reshape(HALF, KNN)
        dpn = dp[:, b][:, idx]  # [T, HALF, KNN, 3] gather
        dpm = np.broadcast_to(dp[:, b, r0:r0 + HALF, None, :],
                              (T, HALF, KNN, 3))
        spn = src[b][idx]  # [HALF, KNN, 3] gather
        spm = np.broadcast_to(src[b, r0:r0 + HALF, None, :], (HALF, KNN, 3))
        in_maps_b.append({
            "dpn": np.ascontiguousarray(dpn).reshape(T, HALF * KNN * 3),
            "dpm": np.ascontiguousarray(dpm).reshape(T, HALF * KNN * 3),
            "spn": np.ascontiguousarray(spn).reshape(HALF * KNN * 3),
            "spm": np.ascontiguousarray(spm).reshape(HALF * KNN * 3),
        })
    global _LAST_INMAPS_B
    _LAST_INMAPS_B = in_maps_b
    resB = run_bass_kernel_spmd(ncB, in_maps_b, core_ids=list(range(8)))
    ar = 0.0
    for c in range(8):
        ar += resB.results[c]["arpart"].sum()
    total += ar / B

    return np.float32(total)


# revision 14
# speedup vs baseline: 1.6676x; 1.6676x over previous
"""Trainium2 Bass kernel for nn_Loss_50233937494630 (chamfer+arap+aux loss).

Sharding (8 cores, data-parallel per the hint): core c handles batch b = c//2,
row-half h = c%2 (rows [2048h, 2048h+2048) of N=4096).

Phase A (one SPMD Bass program on all 8 cores):
  - All pairwise-distance maps are computed on the PE via an augmented K=5
    matmul in float32r producing the NEGATED squared-distance map directly:
      lhsT = [p0, p1, p2, |p|^2, 1], rhs = [2q0, 2q1, 2q2, -1, -|q|^2]
      => (lhsT.T @ rhs)[m, n] = -(|p_m - q_n|^2)
  - Chamfer per t: orientation A (rows = dp half, cols = target) gives d1 via a
    fused tensor_tensor_reduce over slab halves (ACT pre-copies one half of
    each PSUM slab to SBUF so the DVE ingests two fresh elements per cycle);
    orientation B (rows = target, cols = dp half) gives d2 partials.
  - 8-NN of source points: source map with per-core ROTATED columns (so the
    self-diagonal sits at static column blocks on every core), diagonal masked
    with -BIG, then DVE max (top-8) + max_index.  sd = sqrt(-v + 1e-5).
  - pd / sp / tran partials with tiny reductions.

Host: combines partial sums/mins across cores and gathers dp[idx] (pure data
movement).  Phase B (tiny kernel): arap loss from gathered neighbour coords.
"""

import sys
from contextlib import ExitStack

import numpy as np

sys.path.insert(0, "/opt/trn_rl_repo")

import concourse.bass as bass  # noqa: E402
import concourse.mybir as mybir  # noqa: E402
import concourse.tile as tile  # noqa: E402
from concourse import bacc  # noqa: E402
from concourse.bass_utils import run_bass_kernel_spmd  # noqa: E402
from concourse import bass2jax  # noqa: E402


def _make_runner(nc, n_cores=8):
    """Like bass2jax.run_bass_via_pjrt but with the jitted executable built
    once and cached, so repeat calls skip retracing."""
    import jax
    from jax.sharding import Mesh, PartitionSpec
    from jax.experimental.shard_map import shard_map

    bass2jax.install_neuronx_cc_hook()
    partition_name = nc.partition_id_tensor.name if nc.partition_id_tensor else None
    in_names, out_names, out_avals, zero_shapes = [], [], [], []
    for alloc in nc.m.functions[0].allocations:
        if not isinstance(alloc, mybir.MemoryLocationSet):
            continue
        name = alloc.memorylocations[0].name
        if alloc.kind == "ExternalInput":
            if name != partition_name:
                in_names.append(name)
        elif alloc.kind == "ExternalOutput":
            out_names.append(name)
            shape = tuple(alloc.tensor_shape)
            dtype = mybir.dt.np(alloc.dtype)
            out_avals.append(jax.core.ShapedArray(shape, dtype))
            zero_shapes.append((shape, dtype))
    n_params = len(in_names)
    n_outs = len(out_avals)
    all_in_names = list(in_names) + list(out_names)
    if partition_name is not None:
        all_in_names.append(partition_name)
    donate = tuple(range(n_params, n_params + n_outs))

    def _body(*args):
        operands = list(args)
        if partition_name is not None:
            operands.append(bass2jax.partition_id_tensor())
        outs = bass2jax._bass_exec_p.bind(
            *operands, out_avals=tuple(out_avals), in_names=tuple(all_in_names),
            out_names=tuple(out_names), lowering_input_output_aliases=(),
            sim_require_finite=True, sim_require_nnan=True, nc=nc,
        )
        return tuple(outs)

    devices = jax.devices()[:n_cores]
    mesh = Mesh(np.asarray(devices), ("core",))
    sharded = jax.jit(
        shard_map(_body, mesh=mesh,
                  in_specs=(PartitionSpec("core"),) * (n_params + n_outs),
                  out_specs=(PartitionSpec("core"),) * n_outs, check_rep=False),
        donate_argnums=donate, keep_unused=True,
    )

    def run(in_maps):
        concat_in = [
            np.concatenate([np.asarray(in_maps[c][nm]) for c in range(n_cores)],
                           axis=0)
            for nm in in_names
        ]
        concat_zeros = [np.zeros((n_cores * sh[0], *sh[1:]), dt)
                        for sh, dt in zero_shapes]
        out_arrs = sharded(*concat_in, *concat_zeros)
        return [
            {nm: np.asarray(out_arrs[i]).reshape(n_cores, *out_avals[i].shape)[c]
             for i, nm in enumerate(out_names)}
            for c in range(n_cores)
        ]

    return run

T, B, N = 3, 4, 4096
HALF = N // 2
KNN = 8
NT_H = HALF // 128  # 16 row-tiles per core
NT_F = N // 128  # 32 row-tiles over a full point set
NCHUNK = 512
BIG = 1e30
F32 = mybir.dt.float32
MAPDT = mybir.dt.float32r  # PE map matmul dtype (1 cyc/row at N=512)
AluOp = mybir.AluOpType
Act = mybir.ActivationFunctionType
AxX = mybir.AxisListType.X

_CACHE = {}
_LAST_INMAPS_A = None
_LAST_INMAPS_B = None


def _r(ap):
    if MAPDT == F32:
        return ap
    return ap.bitcast(MAPDT)


def _build_planes(nc, pool, raw_ap, npts, name):
    """raw_ap: DRAM [npts, 3] f32 -> DRAM plane [10, npts]:
    rows 0-2 coords, 3 |p|^2, 4 ones  (lhsT form = rows 0:5)
    rows 5-7 2*coords, 8 -1, 9 -|p|^2 (rhs form = rows 5:10)"""
    npart = npts // 32
    pp = pool.tile([npart, 96], F32, tag="ppload", name=f"pp_{name}")
    nc.sync.dma_start(pp[:], raw_ap.rearrange("(p j) c -> p (j c)", j=32))
    f = pool.tile([npart, 320], F32, tag="fbuild", name=f"f_{name}")
    fv = f[:].rearrange("p (f j) -> p f j", j=32)
    ppv = pp[:].rearrange("p (j c) -> p j c", c=3)
    for c in range(3):
        nc.vector.tensor_scalar(fv[:, c, :], ppv[:, :, c], 1.0, None, op0=AluOp.mult)
        nc.vector.tensor_scalar(fv[:, 5 + c, :], ppv[:, :, c], 2.0, None,
                                op0=AluOp.mult)
    sq = pool.tile([npart, 96], F32, tag="sqbuild", name=f"sq_{name}")
    nc.vector.tensor_tensor(sq[:], pp[:], pp[:], op=AluOp.mult)
    nc.vector.tensor_reduce(fv[:, 3, :], sq[:].rearrange("p (j c) -> p j c", c=3),
                            axis=AxX, op=AluOp.add)
    nc.vector.memset(fv[:, 4, :], 1.0)
    nc.vector.memset(fv[:, 8, :], -1.0)
    nc.vector.tensor_scalar(fv[:, 9, :], fv[:, 3, :], -1.0, None, op0=AluOp.mult)
    pl_dram = nc.dram_tensor(f"pldram_{name}", [10, npts], F32,
                             kind="Internal").ap()
    nc.sync.dma_start(pl_dram.rearrange("f (p j) -> p f j", j=32), fv)
    return pl_dram


def _lhs_tile(nc, pool, pl_dram, npts, tag):
    t = pool.tile([5, npts], F32, tag=tag, name=tag)
    nc.sync.dma_start(t[:], pl_dram[0:5, :])
    return t


def _rhs_tile(nc, pool, pl_dram, npts, tag):
    t = pool.tile([5, npts], F32, tag=tag, name=tag)
    nc.sync.dma_start(t[:], pl_dram[5:10, :])
    return t


def _map_tile(nc, psum, lhsT_slice, rhs_tile, col0, ncols, exact=False):
    """matmul a [128, ncols] negated-distance slab into a fresh PSUM tile"""
    cast = (lambda ap: ap) if exact else _r
    slab = psum.tile([128, ncols], F32, tag="slab", name="slab")
    for j in range(ncols // NCHUNK):
        nc.tensor.matmul(
            slab[:, j * NCHUNK:(j + 1) * NCHUNK],
            cast(lhsT_slice),
            cast(rhs_tile[:, col0 + j * NCHUNK:col0 + (j + 1) * NCHUNK]),
            start=True, stop=True,
        )
    return slab


def _build_kernel_a():
    nc = bacc.Bacc("TRN2", target_bir_lowering=False, debug=False)
    src_rot = nc.dram_tensor("src_rot", [N, 3], F32, kind="ExternalInput").ap()
    src_half = nc.dram_tensor("src_half", [HALF, 3], F32, kind="ExternalInput").ap()
    tgt_full = nc.dram_tensor("tgt_full", [N, 3], F32, kind="ExternalInput").ap()
    dp_half = nc.dram_tensor("dp_half", [T, HALF, 3], F32, kind="ExternalInput").ap()
    drp_half = nc.dram_tensor("drp_half", [T, HALF, 3], F32,
                              kind="ExternalInput").ap()
    pw_half = nc.dram_tensor("pw_half", [T, HALF], F32, kind="ExternalInput").ap()
    rigid3 = nc.dram_tensor("rigid3", [1, T * 3], F32, kind="ExternalInput").ap()

    o_d1 = nc.dram_tensor("d1sums", [128, T], F32, kind="ExternalOutput").ap()
    o_d2 = nc.dram_tensor("d2part", [128, T * NT_F], F32, kind="ExternalOutput").ap()
    o_sd = nc.dram_tensor("sd", [128, NT_H * KNN], F32, kind="ExternalOutput").ap()
    o_idx = nc.dram_tensor("idx", [128, NT_H * KNN], mybir.dt.uint32,
                           kind="ExternalOutput").ap()
    o_pd = nc.dram_tensor("pdpart", [128, T], F32, kind="ExternalOutput").ap()
    o_sp = nc.dram_tensor("sppart", [128, T], F32, kind="ExternalOutput").ap()
    o_tr = nc.dram_tensor("tranpart", [1, T], F32, kind="ExternalOutput").ap()

    with tile.TileContext(nc) as tc, ExitStack() as ctx:
        aug = ctx.enter_context(tc.tile_pool(name="aug", bufs=1))
        psum = ctx.enter_context(tc.tile_pool(name="psum", bufs=2, space="PSUM"))

        # ---------- stage 0: augmented planes & operand tiles ----------
        with tc.tile_pool(name="planes", bufs=1) as plp:
            pl_srcF = _build_planes(nc, plp, src_rot, N, "srcF")
            pl_srcH = _build_planes(nc, plp, src_half, HALF, "srcH")
            pl_tgt = _build_planes(nc, plp, tgt_full, N, "tgt")
            pl_dp = [_build_planes(nc, plp, dp_half[t], HALF, f"dp{t}")
                     for t in range(T)]

            lhs_src = _lhs_tile(nc, aug, pl_srcH, HALF, "lhs_src")
            rhs_src = _rhs_tile(nc, aug, pl_srcF, N, "rhs_src")
            lhs_tgt = _lhs_tile(nc, aug, pl_tgt, N, "lhs_tgt")
            rhs_tgt = _rhs_tile(nc, aug, pl_tgt, N, "rhs_tgt")
            lhs_dp = [_lhs_tile(nc, aug, pl_dp[t], HALF, f"lhs_dp{t}")
                      for t in range(T)]
            rhs_dp = [_rhs_tile(nc, aug, pl_dp[t], HALF, f"rhs_dp{t}")
                      for t in range(T)]

        work = ctx.enter_context(tc.tile_pool(name="work", bufs=2))
        accp = ctx.enter_context(tc.tile_pool(name="accs", bufs=1))
        consts = ctx.enter_context(tc.tile_pool(name="consts", bufs=1))
        negbig_full = consts.tile([128, 128], F32)
        nc.vector.memset(negbig_full[:], -BIG)
        diag = consts.tile([128, 128], F32)
        nc.gpsimd.affine_select(
            diag[:], negbig_full[:], pattern=[[-1, 128]],
            compare_op=AluOp.is_equal, fill=0.0, base=0, channel_multiplier=1,
        )

        d1buf = accp.tile([128, T * NT_H], F32)
        d2buf = accp.tile([128, T * NT_F], F32)
        v8buf = accp.tile([128, NT_H * KNN], F32)
        idxbuf = accp.tile([128, NT_H * KNN], mybir.dt.uint32)

        # ---------- stage 1: spsp map + top-8 ----------
        for g in range(NT_H):
            lw = lhs_src[:, g * 128:(g + 1) * 128]
            ssb = work.tile([128, N], F32, tag="ssb", name="ssb")
            for half in range(2):
                slab = _map_tile(nc, psum, lw, rhs_src, half * HALF, HALF,
                                 exact=True)
                nc.scalar.copy(ssb[:, half * HALF:(half + 1) * HALF], slab[:])
            # self-exclusion: with rotated cols, row p of tile g selfs at col 128g+p
            nc.vector.tensor_tensor(
                ssb[:, g * 128:(g + 1) * 128],
                ssb[:, g * 128:(g + 1) * 128], diag[:], op=AluOp.add,
            )
            nc.vector.max(v8buf[:, g * KNN:(g + 1) * KNN], ssb[:])
            nc.vector.max_index(
                idxbuf[:, g * KNN:(g + 1) * KNN],
                v8buf[:, g * KNN:(g + 1) * KNN], ssb[:],
            )
        sdbuf = accp.tile([128, NT_H * KNN], F32)
        epst = consts.tile([128, 1], F32)
        nc.vector.memset(epst[:], 1e-5)
        vcl = accp.tile([128, NT_H * KNN], F32)
        nc.vector.tensor_scalar(vcl[:], v8buf[:], 0.0, None, op0=AluOp.min)
        nc.scalar.activation(sdbuf[:], vcl[:], Act.Sqrt, bias=epst[:], scale=-1.0)
        nc.sync.dma_start(o_sd, sdbuf[:])
        nc.sync.dma_start(o_idx, idxbuf[:])

        # ---------- stage 2: chamfer (maps hold -d; min d == -max(-d)) ----------
        for t in range(T):
            # orientation A: rows = dp half -> d1 row-maxes
            for g in range(NT_H):
                lw = lhs_dp[t][:, g * 128:(g + 1) * 128]
                acc2 = work.tile([128, 2], F32, tag="acc2", name="acc2")
                for half in range(2):
                    slab = _map_tile(nc, psum, lw, rhs_tgt, half * HALF, HALF)
                    nc.vector.tensor_reduce(acc2[:, half:half + 1], slab[:],
                                            axis=AxX, op=AluOp.max)
                nc.vector.tensor_tensor(
                    d1buf[:, t * NT_H + g:t * NT_H + g + 1],
                    acc2[:, 0:1], acc2[:, 1:2], op=AluOp.max,
                )
            # orientation B: rows = target -> d2 partials (min over our dp half)
            for g in range(NT_F):
                lw = lhs_tgt[:, g * 128:(g + 1) * 128]
                slab = _map_tile(nc, psum, lw, rhs_dp[t], 0, HALF)
                nc.vector.tensor_reduce(d2buf[:, t * NT_F + g:t * NT_F + g + 1],
                                        slab[:], axis=AxX, op=AluOp.max)

        nd1 = accp.tile([128, T * NT_H], F32)
        nc.vector.tensor_scalar(nd1[:], d1buf[:], -1.0, None, op0=AluOp.mult)
        d1s = accp.tile([128, T], F32)
        nc.vector.tensor_reduce(d1s[:], nd1[:].rearrange("p (t g) -> p t g", g=NT_H),
                                axis=AxX, op=AluOp.add)
        nc.sync.dma_start(o_d1, d1s[:])
        nd2 = accp.tile([128, T * NT_F], F32)
        nc.vector.tensor_scalar(nd2[:], d2buf[:], -1.0, None, op0=AluOp.mult)
        nc.sync.dma_start(o_d2, nd2[:])

        # ---------- stage 3: small terms ----------
        small = ctx.enter_context(tc.tile_pool(name="small", bufs=1))
        pdbuf = small.tile([128, T], F32)
        spbuf = small.tile([128, T], F32)
        for t in range(T):
            a = small.tile([128, 48], F32, tag="pd_a", name="pd_a")
            bb = small.tile([128, 48], F32, tag="pd_b", name="pd_b")
            nc.sync.dma_start(a[:], drp_half[t].rearrange("(p j) c -> p (j c)", p=128))
            nc.sync.dma_start(bb[:], dp_half[t].rearrange("(p j) c -> p (j c)", p=128))
            dterm = small.tile([128, 48], F32, tag="pd_d", name="pd_d")
            nc.vector.tensor_tensor(dterm[:], a[:], bb[:], op=AluOp.subtract)
            dsq = small.tile([128, 48], F32, tag="pd_j", name="pd_j")
            nc.vector.tensor_tensor(dsq[:], dterm[:], dterm[:], op=AluOp.mult)
            nc.vector.tensor_reduce(pdbuf[:, t:t + 1], dsq[:], axis=AxX,
                                    op=AluOp.add)
            pwt = small.tile([128, 16], F32, tag="pw", name="pw")
            nc.sync.dma_start(pwt[:], pw_half[t].rearrange("(p j) -> p j", p=128))
            nc.vector.tensor_reduce(spbuf[:, t:t + 1], pwt[:], axis=AxX,
                                    op=AluOp.add, apply_absolute_value=True)
        nc.sync.dma_start(o_pd, pdbuf[:])
        nc.sync.dma_start(o_sp, spbuf[:])
        rg = small.tile([1, T * 3], F32)
        nc.sync.dma_start(rg[:], rigid3)
        rsq = small.tile([1, T * 3], F32)
        nc.vector.tensor_tensor(rsq[:], rg[:], rg[:], op=AluOp.mult)
        trb = small.tile([1, T], F32)
        nc.vector.tensor_reduce(trb[:], rsq[:].rearrange("p (t c) -> p t c", c=3),
                                axis=AxX, op=AluOp.add)
        nc.sync.dma_start(o_tr, trb[:])

    nc.compile()
    return nc


def _build_kernel_b():
    nc = bacc.Bacc("TRN2", target_bir_lowering=False, debug=False)
    dpn = nc.dram_tensor("dpn", [T, HALF * KNN * 3], F32, kind="ExternalInput").ap()
    dpm = nc.dram_tensor("dpm", [T, HALF * KNN * 3], F32, kind="ExternalInput").ap()
    spn = nc.dram_tensor("spn", [HALF * KNN * 3], F32, kind="ExternalInput").ap()
    spm = nc.dram_tensor("spm", [HALF * KNN * 3], F32, kind="ExternalInput").ap()
    o_ar = nc.dram_tensor("arpart", [128, T], F32, kind="ExternalOutput").ap()
    FW = NT_H * KNN * 3  # 384 per partition per t
    with tile.TileContext(nc) as tc, ExitStack() as ctx:
        pool = ctx.enter_context(tc.tile_pool(name="p", bufs=1))
        arbuf = pool.tile([128, T], F32)
        epst = pool.tile([128, 1], F32)
        nc.vector.memset(epst[:], 1e-5)
        # sd from gathered source coords, matching the reference computation
        snt = pool.tile([128, FW], F32)
        nc.sync.dma_start(snt[:], spn.rearrange("(p w) -> p w", p=128))
        smt = pool.tile([128, FW], F32)
        nc.sync.dma_start(smt[:], spm.rearrange("(p w) -> p w", p=128))
        sn = pool.tile([128, FW], F32)
        nc.vector.tensor_tensor(sn[:], snt[:], smt[:], op=AluOp.subtract)
        sn2 = pool.tile([128, FW], F32)
        nc.vector.tensor_tensor(sn2[:], sn[:], sn[:], op=AluOp.mult)
        ssum = pool.tile([128, NT_H * KNN], F32)
        nc.vector.tensor_reduce(ssum[:], sn2[:].rearrange("p (w c) -> p w c", c=3),
                                axis=AxX, op=AluOp.add)
        sdt = pool.tile([128, NT_H * KNN], F32)
        nc.scalar.activation(sdt[:], ssum[:], Act.Sqrt, bias=epst[:], scale=1.0)
        for t in range(T):
            nt = pool.tile([128, FW], F32, tag="nt", name="nt")
            nc.sync.dma_start(nt[:], dpn[t].rearrange("(p w) -> p w", p=128))
            mt = pool.tile([128, FW], F32, tag="mt", name="mt")
            nc.sync.dma_start(mt[:], dpm[t].rearrange("(p w) -> p w", p=128))
            dn = pool.tile([128, FW], F32, tag="dn", name="dn")
            nc.vector.tensor_tensor(dn[:], nt[:], mt[:], op=AluOp.subtract)
            dn2 = pool.tile([128, FW], F32, tag="dn2", name="dn2")
            nc.vector.tensor_tensor(dn2[:], dn[:], dn[:], op=AluOp.mult)
            ss = pool.tile([128, NT_H * KNN], F32, tag="ss", name="ss")
            nc.vector.tensor_reduce(ss[:], dn2[:].rearrange("p (w c) -> p w c", c=3),
                                    axis=AxX, op=AluOp.add)
            dd = pool.tile([128, NT_H * KNN], F32, tag="dd", name="dd")
            nc.scalar.activation(dd[:], ss[:], Act.Sqrt, bias=epst[:], scale=1.0)
            df = pool.tile([128, NT_H * KNN], F32, tag="df", name="df")
            nc.vector.tensor_tensor(df[:], dd[:], sdt[:], op=AluOp.subtract)
            df2 = pool.tile([128, NT_H * KNN], F32, tag="dj", name="dj")
            nc.vector.tensor_tensor(df2[:], df[:], df[:], op=AluOp.mult)
            nc.vector.tensor_reduce(arbuf[:, t:t + 1], df2[:], axis=AxX,
                                    op=AluOp.add)
        nc.sync.dma_start(o_ar, arbuf[:])
    nc.compile()
    return nc


def kernel(phi, point_weight, deform_rigid_points, deformation_points,
           rigid_matrix, source_points, target_points):
    pw = np.asarray(point_weight, dtype=np.float32).reshape(T, B, N)
    drp = np.asarray(deform_rigid_points, dtype=np.float32)
    dp = np.asarray(deformation_points, dtype=np.float32)
    rmx = np.asarray(rigid_matrix, dtype=np.float32)
    src = np.asarray(source_points, dtype=np.float32)
    tgt = np.asarray(target_points, dtype=np.float32)

    if "A" not in _CACHE:
        _CACHE["A"] = _build_kernel_a()
        _CACHE["runA"] = _make_runner(_CACHE["A"])
    ncA = _CACHE["A"]

    in_maps = []
    for c in range(8):
        b, h = c // 2, c % 2
        r0 = HALF * h
        in_maps.append({
            "src_rot": np.ascontiguousarray(np.roll(src[b], -r0, axis=0)),
            "src_half": np.ascontiguousarray(src[b, r0:r0 + HALF]),
            "tgt_full": np.ascontiguousarray(tgt[b]),
            "dp_half": np.ascontiguousarray(dp[:, b, r0:r0 + HALF]),
            "drp_half": np.ascontiguousarray(drp[:, b, r0:r0 + HALF]),
            "pw_half": np.ascontiguousarray(pw[:, b, r0:r0 + HALF]),
            "rigid3": np.ascontiguousarray(rmx[:, b, 0:3, 3].reshape(1, T * 3)),
        })
    global _LAST_INMAPS_A
    _LAST_INMAPS_A = in_maps
    outsA = _CACHE["runA"](in_maps)

    d1 = np.zeros(T, dtype=np.float64)
    d2 = np.zeros(T, dtype=np.float64)
    pd = 0.0
    sp = np.zeros(T, dtype=np.float64)
    tran = 0.0
    for b in range(B):
        c0, c1 = outsA[2 * b], outsA[2 * b + 1]
        d1 += c0["d1sums"].sum(axis=0) + c1["d1sums"].sum(axis=0)
        dm = np.minimum(c0["d2part"], c1["d2part"])  # [128, T*32]
        d2 += dm.reshape(128, T, NT_F).sum(axis=(0, 2))
        pd += c0["pdpart"].sum() + c1["pdpart"].sum()
        sp += c0["sppart"].sum(axis=0) + c1["sppart"].sum(axis=0)
        tran += c0["tranpart"].sum()  # h=0 core only
    total = ((d1 + d2) * 0.5 / B).sum() + pd / B + (sp / (B * N)).sum() + tran / B

    # ---- phase B: arap (host gather = data movement only) ----
    if "B" not in _CACHE:
        _CACHE["B"] = _build_kernel_b()
        _CACHE["runB"] = _make_runner(_CACHE["B"])
    ncB = _CACHE["B"]
    in_maps_b = []
    for c in range(8):
        b, h = c // 2, c % 2
        r0 = HALF * h
        idx = outsA[c]["idx"].reshape(128, NT_H, KNN).transpose(1, 0, 2) \
            .reshape(HALF, KNN).astype(np.int64)
        idx = (idx + r0) % N  # un-rotate column space
        sd = outsA[c]["sd"].reshape(128, NT_H, KNN).transpose(1, 0, 2) \
            .reshape(HALF, KNN)
        dpn = dp[:, b][:, idx]  # [T, HALF, KNN, 3] gather
        dpm = np.broadcast_to(dp[:, b, r0:r0 + HALF, None, :],
                              (T, HALF, KNN, 3))
        spn = src[b][idx]  # [HALF, KNN, 3] gather
        spm = np.broadcast_to(src[b, r0:r0 + HALF, None, :], (HALF, KNN, 3))
        in_maps_b.append({
            "dpn": np.ascontiguousarray(dpn).reshape(T, HALF * KNN * 3),
            "dpm": np.ascontiguousarray(dpm).reshape(T, HALF * KNN * 3),
            "spn": np.ascontiguousarray(spn).reshape(HALF * KNN * 3),
            "spm": np.ascontiguousarray(spm).reshape(HALF * KNN * 3),
        })
    global _LAST_INMAPS_B
    _LAST_INMAPS_B = in_maps_b
    outsB = _CACHE["runB"](in_maps_b)
    ar = 0.0
    for c in range(8):
        ar += outsB[c]["arpart"].sum()
    total += ar / B

    return np.float32(total)
